# revision 1
# baseline (speedup 1.0000x reference)
"""GVSL loss (NCC + MSE + smoothness) as a distributed Bass kernel on 8 TRN2 cores.

Sharding: batch(2) x depth-quarters(4) = 8 shards. Each core computes partial
sums for its 32-deep output slab (with 4-voxel halo for the 9^3 box filter);
the final scalar reductions happen on the host.
"""

import os
import sys

for _p in ("/opt/trn_rl_repo",):
    if _p not in sys.path:
        sys.path.insert(0, _p)

import numpy as np
import ml_dtypes

BF16NP = ml_dtypes.bfloat16

import concourse.bass as bass
import concourse.tile as tile
from concourse import bacc, mybir
from concourse.bass_utils import run_bass_kernel_spmd

F32 = mybir.dt.float32
BF16 = mybir.dt.bfloat16
AF = mybir.ActivationFunctionType
ALU = mybir.AluOpType

HP = 128          # partitions (H axis)
W = 128
D_FULL = 128
DQ = 32           # output depths per core
D_IN = DQ + 8     # slab rows incl. halo
WPAD = 137        # 5 zeros | 128 data | 4 zeros
WOFF = 5
NCHUNK = 2
DC_OUT = DQ // NCHUNK          # 16
DC_IN = DC_OUT + 8             # 24
DCPAD = 26                     # 1 zero | 24 data | 1 zero
DPOFF = 1
FLOW_D = DQ + 1                # 33
WIN3 = 729.0

N_IN = D_IN * WPAD             # 5760
N_CHUNK_IN = DC_IN * WPAD      # 3456
N_CHUNK_HB = DC_IN * W         # 3072  (H-boxed compact, per chunk)
N_DPAD = W * DCPAD             # 4096
N_BOX = W * DC_OUT             # 2048
N_RECON = DQ * W               # 4096
N_FLOW_C = FLOW_D * W          # 4224


# acc_all columns
COL_CC0 = 0          # cc sums -> cols 0..7 (chunk0: 2 slices, chunk1: 4)
COL_MSE = 8
COL_DX = 9           # +c, W-axis diffs (3 channels)
COL_DZ = 12          # +c, D-axis diffs
COL_DY = 16          # +c*8+j, H-axis diffs per psum chunk
ACC_W = 40

_CACHE = {}


def _build_program():
    nc = bacc.Bacc("TRN2", target_bir_lowering=False, debug=False, num_devices=8)

    d_imgsA = nc.dram_tensor("imgsA", [HP, N_IN], F32, kind="ExternalInput").ap()
    d_warped = nc.dram_tensor("warped", [HP, N_IN], F32, kind="ExternalInput").ap()
    d_recon = nc.dram_tensor("recon", [HP, N_RECON], BF16, kind="ExternalInput").ap()
    d_mseA = nc.dram_tensor("mseA", [HP, N_RECON], BF16, kind="ExternalInput").ap()
    d_flow = nc.dram_tensor("flow", [HP, 3 * N_FLOW_C], BF16, kind="ExternalInput").ap()
    d_bandp = nc.dram_tensor("bandp", [HP, HP], F32, kind="ExternalInput").ap()
    d_bandn = nc.dram_tensor("bandn", [HP, HP], F32, kind="ExternalInput").ap()
    d_bidiag = nc.dram_tensor("bidiag", [HP, HP - 1], BF16, kind="ExternalInput").ap()
    d_out = nc.dram_tensor("out", [HP, ACC_W], F32, kind="ExternalOutput").ap()

    from contextlib import ExitStack

    with tile.TileContext(nc) as tc, ExitStack() as es:
        pp = es.enter_context(tc.tile_pool(name="persist", bufs=1))
        fp = es.enter_context(tc.tile_pool(name="flowp", bufs=1))
        fdp = es.enter_context(tc.tile_pool(name="diffp", bufs=1))
        rp = es.enter_context(tc.tile_pool(name="reconp", bufs=1))
        sip = es.enter_context(tc.tile_pool(name="srcI", bufs=1))
        prp = es.enter_context(tc.tile_pool(name="prodp", bufs=1))
        cup = es.enter_context(tc.tile_pool(name="cump", bufs=2))
        dpp = es.enter_context(tc.tile_pool(name="dpadp", bufs=2))
        bxp = es.enter_context(tc.tile_pool(name="boxp", bufs=1))
        scp = es.enter_context(tc.tile_pool(name="scrp", bufs=1))
        psp = es.enter_context(tc.tile_pool(name="psum", bufs=1, space="PSUM"))

        acc = pp.tile([HP, ACC_W], F32, tag="acc", name="acc")[:]
        eps_ap = pp.tile([HP, 1], F32, tag="epsc", name="epsc")[:]
        nc.gpsimd.memset(eps_ap, 1e-5)
        bandp = pp.tile([HP, HP], F32, tag="bandp", name="bandp")[:]
        bandn = pp.tile([HP, HP], F32, tag="bandn", name="bandn")[:]
        bidiag = pp.tile([HP, HP - 1], BF16, tag="bidiag", name="bidiag")[:]
        inJ = pp.tile([HP, N_IN], F32, tag="inJ", name="inJ")[:]
        inI = sip.tile([HP, N_IN], F32, tag="inI", name="inI")[:]

        # input DMAs: first-chunk slab rows first so the scans start early
        NJh = (DC_IN // 2) * WPAD
        NJ0 = DC_IN * WPAD
        nc.sync.dma_start(out=bandp, in_=d_bandp)
        nc.sync.dma_start(out=bandn, in_=d_bandn)
        nc.sync.dma_start(out=bidiag, in_=d_bidiag)
        nc.sync.dma_start(out=inJ[:, 0:NJh], in_=d_imgsA[:, 0:NJh])
        nc.sync.dma_start(out=inJ[:, NJh:NJ0], in_=d_imgsA[:, NJh:NJ0])
        nc.sync.dma_start(out=inI[:, 0:NJ0], in_=d_warped[:, 0:NJ0])
        nc.sync.dma_start(out=inJ[:, NJ0:], in_=d_imgsA[:, NJ0:])
        nc.sync.dma_start(out=inI[:, NJ0:], in_=d_warped[:, NJ0:])
        inJ_r = inJ.rearrange("p (d w) -> p d w", w=WPAD)
        inI_r = inI.rearrange("p (d w) -> p d w", w=WPAD)

        recon = rp.tile([HP, N_RECON], BF16, tag="recon", name="recon")[:]
        mseA = rp.tile([HP, N_RECON], BF16, tag="mseA", name="mseA")[:]
        nc.sync.dma_start(out=recon, in_=d_recon)
        nc.sync.dma_start(out=mseA, in_=d_mseA)
        d_flow_r = d_flow.rearrange("p (c d w) -> p c d w", c=3, w=W)

        def ncc_chunk(ch):
            r0 = ch * DC_OUT
            Jc2 = inJ_r[:, r0 : r0 + DC_IN, :].rearrange("p d w -> p (d w)")
            Ic2 = inI_r[:, r0 : r0 + DC_IN, :].rearrange("p d w -> p (d w)")

            boxes = {}
            for v in ("J", "I", "II", "JJ", "IJ"):
                if v == "J":
                    src2 = Jc2
                elif v == "I":
                    src2 = Ic2
                else:
                    prod = prp.tile([HP, N_CHUNK_IN], F32, tag="prod", name="prod")[:]
                    if v == "II":
                        nc.scalar.activation(prod, Ic2, AF.Square)
                    elif v == "JJ":
                        nc.scalar.activation(prod, Jc2, AF.Square)
                    else:
                        nc.vector.tensor_mul(prod, Ic2, Jc2)
                    src2 = prod

                # W-axis cumsum in two chained halves (box diff is fused
                # into the H-box matmuls via the +/- band pair)
                NH = N_CHUNK_IN // 2
                cum_a = cup.tile([HP, NH], F32, tag="cuma", name="cuma")[:]
                cum_b = cup.tile([HP, NH], F32, tag="cumb", name="cumb")[:]
                nc.vector.tensor_tensor_scan(
                    cum_a, src2[:, 0:NH], src2[:, 0:NH],
                    0.0, op0=ALU.add, op1=ALU.bypass,
                )
                nc.vector.tensor_tensor_scan(
                    cum_b, src2[:, NH:], src2[:, NH:],
                    cum_a[:, NH - 1 : NH], op0=ALU.add, op1=ALU.bypass,
                )
                cum_ar = cum_a.rearrange("p (d w) -> p d w", w=WPAD)
                cum_br = cum_b.rearrange("p (d w) -> p d w", w=WPAD)

                dpad = dpp.tile([HP, N_DPAD], F32, tag="dpad", name="dpad")[:]
                dpad_r = dpad.rearrange("p (w dp) -> p w dp", dp=DCPAD)
                nc.gpsimd.memset(dpad_r[:, :, 0:DPOFF], 0.0)
                nc.gpsimd.memset(dpad_r[:, :, DPOFF + DC_IN : DCPAD], 0.0)

                for j in range(N_CHUNK_HB // 512):
                    dlo = 4 * j
                    ps = psp.tile([HP, 512], F32, tag="ps", name="ps", bufs=6)[:]
                    cr = cum_ar if j < 3 else cum_br
                    dl = dlo if j < 3 else dlo - 12
                    rhs9 = cr[:, dl : dl + 4, 9 : 9 + W]
                    rhs0 = cr[:, dl : dl + 4, 0:W]
                    nc.tensor.matmul(ps, bandp, rhs9, start=True, stop=False)
                    nc.tensor.matmul(ps, bandn, rhs0, start=False, stop=True)
                    ps_wd = ps.rearrange("p (s w) -> p w s", w=W)
                    nc.scalar.copy(
                        dpad_r[:, :, DPOFF + dlo : DPOFF + dlo + 4], ps_wd
                    )

                # D-axis cumsum (in place) + diff -> final 9^3 box sums
                nc.vector.tensor_tensor_scan(
                    dpad, dpad, dpad, 0.0, op0=ALU.add, op1=ALU.bypass
                )
                cumd_r = dpad.rearrange("p (w dp) -> p w dp", dp=DCPAD)
                B = bxp.tile([HP, N_BOX], F32, tag=f"box{v}", name=f"box{v}")[:]
                B_r = B.rearrange("p (w d) -> p w d", d=DC_OUT)
                nc.vector.tensor_sub(
                    B_r,
                    cumd_r[:, :, 9 : 9 + DC_OUT],
                    cumd_r[:, :, 0 : 0 + DC_OUT],
                )
                boxes[v] = B

            # cc math in 2 slices so DVE and ACT pipeline across slices
            NS = N_BOX // 2
            for sl in range(2):
                lo, hi = sl * NS, (sl + 1) * NS
                BJ = boxes["J"][:, lo:hi]
                BI = boxes["I"][:, lo:hi]
                BII = boxes["II"][:, lo:hi]
                BJJ = boxes["JJ"][:, lo:hi]
                BIJ = boxes["IJ"][:, lo:hi]
                s1 = scp.tile([HP, NS], F32, tag="s1", name="s1")[:]
                s2 = scp.tile([HP, NS], F32, tag="s2", name="s2")[:]
                s3 = scp.tile([HP, NS], F32, tag="s3", name="s3")[:]

                nc.vector.tensor_mul(s1, BI, BJ)
                nc.vector.scalar_tensor_tensor(
                    s2, s1, -1.0 / WIN3, BIJ, op0=ALU.mult, op1=ALU.add
                )  # cross
                nc.scalar.activation(s1, s2, AF.Square)   # cross^2
                nc.scalar.activation(s2, s1, AF.Ln)       # ln(cross^2)
                nc.scalar.activation(s1, BI, AF.Square)
                nc.vector.scalar_tensor_tensor(
                    s3, s1, -1.0 / WIN3, BII, op0=ALU.mult, op1=ALU.add
                )  # I_var
                nc.scalar.activation(s1, BJ, AF.Square)
                nc.vector.scalar_tensor_tensor(
                    BII, s1, -1.0 / WIN3, BJJ, op0=ALU.mult, op1=ALU.add
                )  # J_var (overwrites dead BII)
                nc.vector.tensor_mul(s1, s3, BII)         # I_var * J_var
                nc.scalar.activation(s3, s1, AF.Ln, bias=eps_ap)
                nc.vector.tensor_sub(s1, s2, s3)
                col = COL_CC0 + ch * 2 + sl
                nc.scalar.activation(
                    s3, s1, AF.Exp, accum_out=acc[:, col : col + 1]
                )

        def flow_compute():
            # MSE
            mbuf = fdp.tile([HP, N_RECON], BF16, tag="dbuf", name="dbuf")[:]
            nc.vector.tensor_sub(mbuf, mseA, recon)
            nc.scalar.activation(
                mbuf, mbuf, AF.Square, accum_out=acc[:, COL_MSE : COL_MSE + 1]
            )
            for c in range(3):
                fc = fp.tile([HP, N_FLOW_C], BF16, tag="fc", name="fc", bufs=2)[:]
                nc.sync.dma_start(
                    out=fc, in_=d_flow_r[:, c].rearrange("p d w -> p (d w)")
                )
                fc_r = fc.rearrange("p (d w) -> p d w", w=W)

                # W-axis diffs (innermost)
                db = fdp.tile([HP, N_RECON], BF16, tag="dbuf", name="dbuf")[:]
                db_x = db.rearrange("p (d w) -> p d w", w=W)[:, :, 0 : W - 1]
                nc.vector.tensor_sub(
                    db_x, fc_r[:, 0:DQ, 1:W], fc_r[:, 0:DQ, 0 : W - 1]
                )
                col = COL_DX + c
                nc.scalar.activation(
                    db.rearrange("p (d w) -> p d w", w=W)[:, :, 0 : W - 1],
                    db.rearrange("p (d w) -> p d w", w=W)[:, :, 0 : W - 1],
                    AF.Square,
                    accum_out=acc[:, col : col + 1],
                )

                # H-axis diffs on the PE: psum = bidiag^T @ fc, squared in
                # place on PSUM (one acc column per psum chunk)
                fc_flat = fc_r[:, 0:DQ, :].rearrange("p d w -> p (d w)")
                for j in range(N_RECON // 512):
                    ps = psp.tile([HP, 512], F32, tag="fps", name="fps", bufs=2)[:]
                    nc.tensor.matmul(
                        ps[0 : HP - 1, :],
                        bidiag,
                        fc_flat[:, 512 * j : 512 * (j + 1)],
                        start=True,
                        stop=True,
                    )
                    col = COL_DY + c * 8 + j
                    nc.scalar.activation(
                        ps[0 : HP - 1, :],
                        ps[0 : HP - 1, :],
                        AF.Square,
                        accum_out=acc[0 : HP - 1, col : col + 1],
                    )

                # D-axis diffs
                db = fdp.tile([HP, N_RECON], BF16, tag="dbuf", name="dbuf")[:]
                nc.vector.tensor_sub(
                    db,
                    fc_r[:, 1 : DQ + 1, :].rearrange("p d w -> p (d w)"),
                    fc_r[:, 0:DQ, :].rearrange("p d w -> p (d w)"),
                )
                col = COL_DZ + c
                nc.scalar.activation(
                    db, db, AF.Square, accum_out=acc[:, col : col + 1]
                )

        ncc_chunk(0)
        ncc_chunk(1)
        flow_compute()
        nc.sync.dma_start(out=d_out, in_=acc)

    nc.compile()
    return nc


def _make_band() -> tuple[np.ndarray, np.ndarray, np.ndarray]:
    k = np.arange(HP)
    band = (np.abs(k[:, None] - k[None, :]) <= 4).astype(np.float32)
    m = np.arange(HP - 1)
    bidiag = np.zeros((HP, HP - 1), BF16NP)
    bidiag[m + 1, m] = 1.0
    bidiag[m, m] = -1.0
    return band, -band, bidiag


def _shard_inputs(imgsA, recon_A, warped_BA, flow_BA):
    bandp, bandn, bidiag = _make_band()
    in_maps = []
    for core in range(8):
        b, q = divmod(core, 4)
        d0 = DQ * q

        def slab(vol):
            s = np.zeros((HP, D_IN, WPAD), np.float32)
            lo, hi = d0 - 4, d0 + DQ + 4
            clo, chi = max(lo, 0), min(hi, D_FULL)
            s[:, clo - lo : chi - lo, WOFF : WOFF + W] = np.ascontiguousarray(
                vol[clo:chi].transpose(1, 0, 2)
            )
            return s.reshape(HP, N_IN)

        rec = np.ascontiguousarray(
            recon_A[b, 0, d0 : d0 + DQ].transpose(1, 0, 2)
        ).astype(BF16NP).reshape(HP, N_RECON)
        msea = np.ascontiguousarray(
            imgsA[b, 0, d0 : d0 + DQ].transpose(1, 0, 2)
        ).astype(BF16NP).reshape(HP, N_RECON)

        fl = np.empty((HP, 3, FLOW_D, W), BF16NP)
        hi = min(d0 + FLOW_D, D_FULL)
        n = hi - d0
        fl[:, :, :n] = flow_BA[b, :, d0:hi].transpose(2, 0, 1, 3)
        if n < FLOW_D:
            fl[:, :, n:] = fl[:, :, n - 1 : n]

        in_maps.append(
            {
                "imgsA": slab(imgsA[b, 0]),
                "warped": slab(warped_BA[b, 0]),
                "recon": rec,
                "mseA": msea,
                "flow": np.ascontiguousarray(fl).reshape(HP, 3 * N_FLOW_C),
                "bandp": bandp,
                "bandn": bandn,
                "bidiag": bidiag,
            }
        )
    return in_maps


def _install_profile_shim():
    """Wire up NTFF profiling under axon when antenv.axon_hooks is absent."""
    try:
        import antenv.axon_hooks  # noqa: F401

        return True
    except ImportError:
        pass
    import contextlib
    import ctypes
    import types

    so_path = "/opt/axon/libaxon_pjrt.so"
    if not os.path.exists(so_path):
        return False
    lib = ctypes.CDLL(so_path)
    if not hasattr(lib, "axon_start_nrt_profile"):
        return False
    lib.axon_start_nrt_profile.argtypes = [
        ctypes.POINTER(ctypes.c_int64),
        ctypes.c_size_t,
    ]
    lib.axon_start_nrt_profile.restype = ctypes.c_int64
    lib.axon_stop_nrt_profile.argtypes = [ctypes.c_char_p]
    lib.axon_stop_nrt_profile.restype = ctypes.c_int64

    @contextlib.contextmanager
    def _hook(output_dir, device_ids):
        import jax

        jax.devices()
        if device_ids:
            ids = (ctypes.c_int64 * len(device_ids))(*device_ids)
            rc = lib.axon_start_nrt_profile(ids, len(device_ids))
        else:
            rc = lib.axon_start_nrt_profile(None, 0)
        if rc != 0:
            raise RuntimeError(f"axon_start_nrt_profile rc={rc}")
        try:
            yield
        finally:
            n = lib.axon_stop_nrt_profile(str(output_dir).encode())
            print(f"ntff profile: {n} file(s) written to {output_dir}")

    mod = types.ModuleType("antenv.axon_hooks")
    mod.get_axon_ntff_profile_hook = lambda: _hook
    mod.set_axon_ntff_profile_hook = lambda h: None
    import antenv

    sys.modules["antenv.axon_hooks"] = mod
    antenv.axon_hooks = mod

    # keep profile artifacts local instead of uploading to fishnet
    import concourse.bass_utils as _bu

    _bu.upload_artifacts = lambda tmpdir: tmpdir
    return True


LAST_EXEC_NS = None
LAST_RESULTS = None


def kernel(imgsA, recon_A, warped_BA, flow_BA):
    global LAST_EXEC_NS, LAST_RESULTS
    if "nc" not in _CACHE:
        _CACHE["nc"] = _build_program()
    nc = _CACHE["nc"]

    in_maps = _shard_inputs(
        np.asarray(imgsA, np.float32),
        np.asarray(recon_A, np.float32),
        np.asarray(warped_BA, np.float32),
        np.asarray(flow_BA, np.float32),
    )
    trace = os.environ.get("GVSL_TRACE", "0") == "1"
    if trace:
        trace = _install_profile_shim()
    tmpdir = os.environ.get("GVSL_TRACE_DIR") or None
    res = run_bass_kernel_spmd(
        nc, in_maps, core_ids=list(range(8)), trace=trace, tmpdir=tmpdir
    )
    LAST_EXEC_NS = res.exec_time_ns
    LAST_RESULTS = res

    cc = mse = dx = dy = dz = 0.0
    for r in res.results:
        o = np.asarray(r["out"], np.float64)
        cc += o[:, COL_CC0 : COL_CC0 + 4].sum()
        mse += o[:, COL_MSE].sum()
        dx += o[:, COL_DX : COL_DX + 3].sum()
        dy += o[: HP - 1, COL_DY : COL_DY + 24].sum()
        dz += o[:, COL_DZ : COL_DZ + 3].sum()

    n_vox = 2 * 1 * 128 * 128 * 128
    n_d = 2 * 3 * 127 * 128 * 128
    ncc_loss = 1.0 - cc / n_vox
    mse_loss = mse / n_vox
    smooth_loss = (dx / n_d + dy / n_d + dz / n_d) / 3.0
    return (
        np.float32(ncc_loss),
        np.float32(mse_loss),
        np.float32(smooth_loss),
    )



# revision 7
# speedup vs baseline: 1.4844x; 1.4844x over previous
"""GVSL loss (NCC + MSE + smoothness) as a distributed Bass kernel on 8 TRN2 cores.

Sharding: batch(2) x depth-quarters(4) = 8 shards; each core owns a 32-deep
output slab (+4-voxel halo for the 9^3 box filter).

NCC box filter strategy (per var in {I, J, I^2, J^2, IJ}):
  pass1 (PE):  per d-row matmul(lhsT=V_d[h,w], rhs=BandH[h,h']) -> PSUM [w, h]
               = H-box + transpose in one shot (fp16, FD=128, LDW pipelined)
  evac1:       PSUM -> SBUF fp16 YT [w, (d, h)]
  pass2 (PE):  stationary BandW*(1/27); 3 d-shifted FD=512 matmuls accumulate
               -> t3[r] = Z[r]+Z[r+1]+Z[r+2] (W-box + D-triple), PSUM
  evac2:       PSUM -> SBUF fp16 T3 [w', (r, h)]
  D-final(DVE): S = t3[d] + t3[d+3] + t3[d+6]  (9-window box done)
All box values carry a 1/27 scale so fp16 never overflows in the cc math;
the scale cancels in cc except for eps, which is scaled accordingly.
"""

import os
import sys

for _p in ("/opt/trn_rl_repo",):
    if _p not in sys.path:
        sys.path.insert(0, _p)

import numpy as np

import concourse.bass as bass
import concourse.tile as tile
from concourse import bacc, mybir
from concourse.bass_utils import run_bass_kernel_spmd

F32 = mybir.dt.float32
F16 = mybir.dt.float16
AF = mybir.ActivationFunctionType
ALU = mybir.AluOpType

HP = 128          # partitions (H axis in input layout)
W = 128
D_FULL = 128
DQ = 32           # output depths per core
D_IN = DQ + 8     # slab rows incl halo (40)
D_PAD = 44        # YT rows incl zero tail so pass2 shifted reads stay in range
T3R = 38          # valid t3 rows
FLOW_D = DQ + 1   # 33

N_IN = D_IN * W           # 5120 per partition (d-major, w-inner)
N_YT = D_PAD * HP         # 5632  [w, (d, h)]
N_BOX = DQ * HP           # 4096  [w, (do, h)]
N_RECON = DQ * W          # 4096
N_FLOW_C = FLOW_D * W     # 4224

S16 = float(np.float16(1.0 / 27.0))   # box scale applied via BandW
CINV = 1.0 / (729.0 * S16)            # cross = B'IJ - CINV * B'I*B'J
EPS_S = 1e-5 * S16 * S16              # eps matching the scaled denom

# acc columns
COL_CC = 0     # +2
COL_MSE = 2
COL_DX = 3     # +3 (per channel)
COL_DZ = 6     # +3
COL_DY = 9     # +12 (4 per channel)
ACC_W = 24

_CACHE = {}


def _build_program():
    nc = bacc.Bacc("TRN2", target_bir_lowering=False, debug=False, num_devices=8)

    d_inI = nc.dram_tensor("inI", [HP, N_IN], F16, kind="ExternalInput").ap()
    d_inJ = nc.dram_tensor("inJ", [HP, N_IN], F16, kind="ExternalInput").ap()
    d_recon = nc.dram_tensor("recon", [HP, N_RECON], F16, kind="ExternalInput").ap()
    d_flow = nc.dram_tensor("flow", [HP, 3 * N_FLOW_C], F16, kind="ExternalInput").ap()
    d_bandh = nc.dram_tensor("bandh", [HP, HP], F16, kind="ExternalInput").ap()
    d_bandw = nc.dram_tensor("bandw", [HP, HP], F16, kind="ExternalInput").ap()
    d_bidiag = nc.dram_tensor("bidiag", [HP, HP - 1], F16, kind="ExternalInput").ap()
    d_out = nc.dram_tensor("out", [HP, ACC_W], F32, kind="ExternalOutput").ap()

    from contextlib import ExitStack

    with tile.TileContext(nc) as tc, ExitStack() as es:
        pp = es.enter_context(tc.tile_pool(name="persist", bufs=1))
        prp = es.enter_context(tc.tile_pool(name="prodp", bufs=2))
        ytp = es.enter_context(tc.tile_pool(name="ytp", bufs=2))
        t3p = es.enter_context(tc.tile_pool(name="t3p", bufs=2))
        bxp = es.enter_context(tc.tile_pool(name="boxp", bufs=1))
        scp = es.enter_context(tc.tile_pool(name="scrp", bufs=1))
        flp = es.enter_context(tc.tile_pool(name="flowscr", bufs=2))
        ps1 = es.enter_context(tc.tile_pool(name="psum1", bufs=2, space="PSUM"))
        ps2 = es.enter_context(tc.tile_pool(name="psum2", bufs=2, space="PSUM"))
        ps3 = es.enter_context(tc.tile_pool(name="psum3", bufs=1, space="PSUM"))

        acc = pp.tile([HP, ACC_W], F32, tag="acc", name="acc")[:]
        eps_ap = pp.tile([HP, 1], F32, tag="epsc", name="epsc")[:]
        nc.gpsimd.memset(eps_ap, EPS_S)

        bandh = pp.tile([HP, HP], F16, tag="bandh", name="bandh")[:]
        bandw = pp.tile([HP, HP], F16, tag="bandw", name="bandw")[:]
        bidiag = pp.tile([HP, HP - 1], F16, tag="bidiag", name="bidiag")[:]
        inI = pp.tile([HP, N_IN], F16, tag="inI", name="inI")[:]
        inJ = pp.tile([HP, N_IN], F16, tag="inJ", name="inJ")[:]
        recon = pp.tile([HP, N_RECON], F16, tag="recon", name="recon")[:]
        flow = pp.tile([HP, 3 * N_FLOW_C], F16, tag="flow", name="flow")[:]

        nc.sync.dma_start(out=bandh, in_=d_bandh)
        nc.sync.dma_start(out=bandw, in_=d_bandw)
        nc.sync.dma_start(out=bidiag, in_=d_bidiag)
        NH = N_IN // 2
        nc.sync.dma_start(out=inJ[:, 0:NH], in_=d_inJ[:, 0:NH])
        nc.sync.dma_start(out=inJ[:, NH:], in_=d_inJ[:, NH:])
        nc.sync.dma_start(out=inI[:, 0:NH], in_=d_inI[:, 0:NH])
        nc.sync.dma_start(out=inI[:, NH:], in_=d_inI[:, NH:])
        nc.sync.dma_start(out=recon, in_=d_recon)
        nc.sync.dma_start(out=flow, in_=d_flow)

        inI_r = inI.rearrange("p (d w) -> p d w", w=W)
        inJ_r = inJ.rearrange("p (d w) -> p d w", w=W)

        def make_product(kind):
            prod = prp.tile([HP, N_IN], F16, tag="prod", name=f"prod{kind}")[:]
            if kind == "II":
                nc.vector.tensor_mul(prod, inI, inI)
            elif kind == "JJ":
                nc.vector.tensor_mul(prod, inJ, inJ)
            else:
                nc.vector.tensor_mul(prod, inI, inJ)
            return prod.rearrange("p (d w) -> p d w", w=W)

        def ncc_var(vi, src_r):
            """src_r: [h, d(40), w] fp16. Returns box tile B [w, (do, h)] fp16."""
            # --- pass1: H-box + transpose, per d-row ---
            yt = ytp.tile([HP, N_YT], F16, tag="yt", name=f"yt{vi}")[:]
            yt_r = yt.rearrange("p (d h) -> p d h", h=HP)
            # zero tail rows (pass2 shifted reads touch rows 40..43)
            nc.gpsimd.memset(yt_r[:, D_IN:D_PAD, :], 0.0)
            for g in range(5):  # groups of 8 d-rows -> 2 psum banks
                pst = ps1.tile([HP, 1024], F32, tag="ps1", name="ps1")[:]
                for q in range(8):
                    d = 8 * g + q
                    nc.tensor.matmul(
                        pst[:, 128 * q : 128 * (q + 1)],
                        src_r[:, d, :],
                        bandh,
                        start=True,
                        stop=True,
                    )
                dst = yt_r[:, 8 * g : 8 * g + 8, :].rearrange("p d h -> p (d h)")
                if g % 2 == 0:
                    nc.scalar.copy(dst, pst)
                else:
                    nc.vector.tensor_copy(dst, pst)

            # --- pass2: W-box (scaled 1/27) + D-triple via 3 shifted matmuls ---
            t3 = t3p.tile([HP, N_YT], F16, tag="t3", name=f"t3{vi}")[:]
            t3_r = t3.rearrange("p (r h) -> p r h", h=HP)
            for k in range(10):  # t3 rows 4k..4k+3
                pst = ps2.tile([HP, 512], F32, tag="ps2", name="ps2")[:]
                for s in range(3):
                    rhs = yt_r[:, 4 * k + s : 4 * k + s + 4, :].rearrange(
                        "p d h -> p (d h)"
                    )
                    nc.tensor.matmul(
                        pst, bandw, rhs, start=(s == 0), stop=(s == 2)
                    )
                dst = t3_r[:, 4 * k : 4 * k + 4, :].rearrange("p r h -> p (r h)")
                if k % 2 == 0:
                    nc.scalar.copy(dst, pst)
                else:
                    nc.vector.tensor_copy(dst, pst)

            # --- D-final: S[do] = t3[do] + t3[do+3] + t3[do+6] ---
            B = bxp.tile([HP, N_BOX], F16, tag=f"box{vi}", name=f"box{vi}")[:]
            B_r = B.rearrange("p (do h) -> p do h", h=HP)
            nc.vector.tensor_add(B_r, t3_r[:, 0:DQ, :], t3_r[:, 3 : 3 + DQ, :])
            nc.vector.tensor_add(B_r, B_r, t3_r[:, 6 : 6 + DQ, :])
            return B

        BJ = ncc_var(0, inJ_r)
        BI = ncc_var(1, inI_r)
        BII = ncc_var(2, make_product("II"))
        BJJ = ncc_var(3, make_product("JJ"))
        BIJ = ncc_var(4, make_product("IJ"))

        # --- cc math (scaled boxes; scale cancels, eps pre-scaled) ---
        NS = N_BOX // 2
        for sl in range(2):
            lo, hi = sl * NS, (sl + 1) * NS
            s1 = scp.tile([HP, NS], F16, tag="s1", name="s1")[:]
            s2 = scp.tile([HP, NS], F16, tag="s2", name="s2")[:]
            s3 = scp.tile([HP, NS], F16, tag="s3", name="s3")[:]
            bi, bj = BI[:, lo:hi], BJ[:, lo:hi]
            bii, bjj, bij = BII[:, lo:hi], BJJ[:, lo:hi], BIJ[:, lo:hi]

            nc.vector.tensor_mul(s1, bi, bj)
            nc.vector.scalar_tensor_tensor(
                s2, s1, -CINV, bij, op0=ALU.mult, op1=ALU.add
            )  # crossS
            nc.scalar.activation(s1, s2, AF.Square)   # crossS^2
            nc.scalar.activation(s2, s1, AF.Ln)
            nc.vector.tensor_mul(s1, bi, bi)
            nc.vector.scalar_tensor_tensor(
                s3, s1, -CINV, bii, op0=ALU.mult, op1=ALU.add
            )  # IvarS
            nc.vector.tensor_mul(s1, bj, bj)
            nc.vector.scalar_tensor_tensor(
                s1, s1, -CINV, bjj, op0=ALU.mult, op1=ALU.add
            )  # JvarS
            nc.vector.tensor_mul(s1, s1, s3)          # denomS
            nc.scalar.activation(s3, s1, AF.Ln, bias=eps_ap)
            nc.vector.tensor_sub(s1, s2, s3)
            col = COL_CC + sl
            nc.scalar.activation(
                s3, s1, AF.Exp, accum_out=acc[:, col : col + 1]
            )

        # --- MSE: imgsA rows 4..36 of the J slab vs recon ---
        md = flp.tile([HP, N_RECON], F16, tag="fscr", name="mse")[:]
        nc.vector.tensor_sub(
            md,
            inJ_r[:, 4 : 4 + DQ, :].rearrange("p d w -> p (d w)"),
            recon,
        )
        nc.scalar.activation(
            md, md, AF.Square, accum_out=acc[:, COL_MSE : COL_MSE + 1]
        )

        # --- flow smoothness ---
        flow_r = flow.rearrange("p (c d w) -> p c d w", c=3, w=W)
        for c in range(3):
            fc = flow_r[:, c]                       # [p, 33, 128]
            fc_flat = fc.rearrange("p d w -> p (d w)")

            # w-diffs: flat shifted sub (col 127 is wrap garbage; the square
            # skips it via a strided AP)
            db = flp.tile([HP, N_RECON], F16, tag="fscr", name=f"dx{c}")[:]
            nc.vector.tensor_sub(
                db, fc_flat[:, 1 : 1 + N_RECON], fc_flat[:, 0:N_RECON]
            )
            db_r = db.rearrange("p (d w) -> p d w", w=W)
            nc.scalar.activation(
                db_r[:, :, 0 : W - 1],
                db_r[:, :, 0 : W - 1],
                AF.Square,
                accum_out=acc[:, COL_DX + c : COL_DX + c + 1],
            )

            # d-diffs
            db2 = flp.tile([HP, N_RECON], F16, tag="fscr", name=f"dz{c}")[:]
            nc.vector.tensor_sub(
                db2, fc_flat[:, W : W + N_RECON], fc_flat[:, 0:N_RECON]
            )
            nc.scalar.activation(
                db2, db2, AF.Square, accum_out=acc[:, COL_DZ + c : COL_DZ + c + 1]
            )

            # h-diffs on the PE: psum = bidiag^T @ fc rows 0..31
            for half in range(4):
                pst = ps3.tile([HP, 1024], F32, tag="ps3", name="ps3")[:]
                for j in range(2):
                    off = 1024 * half + 512 * j
                    nc.tensor.matmul(
                        pst[0 : HP - 1, 512 * j : 512 * (j + 1)],
                        bidiag,
                        fc_flat[:, off : off + 512],
                        start=True,
                        stop=True,
                    )
                col = COL_DY + 4 * c + half
                nc.scalar.activation(
                    pst[0 : HP - 1, :],
                    pst[0 : HP - 1, :],
                    AF.Square,
                    accum_out=acc[0 : HP - 1, col : col + 1],
                )

        nc.sync.dma_start(out=d_out, in_=acc)

    nc.compile()
    return nc


def _make_consts():
    k = np.arange(HP)
    band = (np.abs(k[:, None] - k[None, :]) <= 4).astype(np.float16)
    bandw = (band * np.float16(S16)).astype(np.float16)
    m = np.arange(HP - 1)
    bidiag = np.zeros((HP, HP - 1), np.float16)
    bidiag[m + 1, m] = 1.0
    bidiag[m, m] = -1.0
    return band, bandw, bidiag


def _shard_inputs(imgsA, recon_A, warped_BA, flow_BA):
    bandh, bandw, bidiag = _make_consts()
    in_maps = []
    for core in range(8):
        b, q = divmod(core, 4)
        d0 = DQ * q

        def slab(vol):
            s = np.zeros((HP, D_IN, W), np.float16)
            lo, hi = d0 - 4, d0 + DQ + 4
            clo, chi = max(lo, 0), min(hi, D_FULL)
            s[:, clo - lo : chi - lo, :] = vol[clo:chi].transpose(1, 0, 2)
            return s.reshape(HP, N_IN)

        rec = (
            recon_A[b, 0, d0 : d0 + DQ]
            .transpose(1, 0, 2)
            .astype(np.float16)
            .reshape(HP, N_RECON)
        )

        fl = np.empty((HP, 3, FLOW_D, W), np.float16)
        hi = min(d0 + FLOW_D, D_FULL)
        n = hi - d0
        fl[:, :, :n] = flow_BA[b, :, d0:hi].transpose(2, 0, 1, 3)
        if n < FLOW_D:
            fl[:, :, n:] = fl[:, :, n - 1 : n]

        in_maps.append(
            {
                "inI": slab(warped_BA[b, 0]),
                "inJ": slab(imgsA[b, 0]),
                "recon": np.ascontiguousarray(rec),
                "flow": np.ascontiguousarray(fl).reshape(HP, 3 * N_FLOW_C),
                "bandh": bandh,
                "bandw": bandw,
                "bidiag": bidiag,
            }
        )
    return in_maps


def _install_profile_shim():
    """Wire up NTFF profiling under axon when antenv.axon_hooks is absent."""
    try:
        import antenv.axon_hooks  # noqa: F401

        return True
    except ImportError:
        pass
    import contextlib
    import ctypes
    import types

    so_path = "/opt/axon/libaxon_pjrt.so"
    if not os.path.exists(so_path):
        return False
    lib = ctypes.CDLL(so_path)
    if not hasattr(lib, "axon_start_nrt_profile"):
        return False
    lib.axon_start_nrt_profile.argtypes = [
        ctypes.POINTER(ctypes.c_int64),
        ctypes.c_size_t,
    ]
    lib.axon_start_nrt_profile.restype = ctypes.c_int64
    lib.axon_stop_nrt_profile.argtypes = [ctypes.c_char_p]
    lib.axon_stop_nrt_profile.restype = ctypes.c_int64

    @contextlib.contextmanager
    def _hook(output_dir, device_ids):
        import jax

        jax.devices()
        if device_ids:
            ids = (ctypes.c_int64 * len(device_ids))(*device_ids)
            rc = lib.axon_start_nrt_profile(ids, len(device_ids))
        else:
            rc = lib.axon_start_nrt_profile(None, 0)
        if rc != 0:
            raise RuntimeError(f"axon_start_nrt_profile rc={rc}")
        try:
            yield
        finally:
            n = lib.axon_stop_nrt_profile(str(output_dir).encode())
            print(f"ntff profile: {n} file(s) written to {output_dir}")

    mod = types.ModuleType("antenv.axon_hooks")
    mod.get_axon_ntff_profile_hook = lambda: _hook
    mod.set_axon_ntff_profile_hook = lambda h: None
    import antenv

    sys.modules["antenv.axon_hooks"] = mod
    antenv.axon_hooks = mod

    import concourse.bass_utils as _bu

    _bu.upload_artifacts = lambda tmpdir: tmpdir
    return True


LAST_EXEC_NS = None
LAST_RESULTS = None


def kernel(imgsA, recon_A, warped_BA, flow_BA):
    global LAST_EXEC_NS, LAST_RESULTS
    if "nc" not in _CACHE:
        _CACHE["nc"] = _build_program()
    nc = _CACHE["nc"]

    in_maps = _shard_inputs(
        np.asarray(imgsA, np.float32),
        np.asarray(recon_A, np.float32),
        np.asarray(warped_BA, np.float32),
        np.asarray(flow_BA, np.float32),
    )
    trace = os.environ.get("GVSL_TRACE", "0") == "1"
    if trace:
        trace = _install_profile_shim()
    tmpdir = os.environ.get("GVSL_TRACE_DIR") or None
    res = run_bass_kernel_spmd(
        nc, in_maps, core_ids=list(range(8)), trace=trace, tmpdir=tmpdir
    )
    LAST_EXEC_NS = res.exec_time_ns
    LAST_RESULTS = res

    cc = mse = dx = dy = dz = 0.0
    for r in res.results:
        o = np.asarray(r["out"], np.float64)
        cc += o[:, COL_CC : COL_CC + 2].sum()
        mse += o[:, COL_MSE].sum()
        dx += o[:, COL_DX : COL_DX + 3].sum()
        dz += o[:, COL_DZ : COL_DZ + 3].sum()
        dy += o[: HP - 1, COL_DY : COL_DY + 12].sum()

    n_vox = 2 * 1 * 128 * 128 * 128
    n_d = 2 * 3 * 127 * 128 * 128
    ncc_loss = 1.0 - cc / n_vox
    mse_loss = mse / n_vox
    smooth_loss = (dx / n_d + dy / n_d + dz / n_d) / 3.0
    return (
        np.float32(ncc_loss),
        np.float32(mse_loss),
        np.float32(smooth_loss),
    )


# revision 12
# speedup vs baseline: 1.8988x; 1.2792x over previous
"""GVSL loss (NCC + MSE + smoothness) as a distributed Bass kernel on 8 TRN2 cores.

Sharding: batch(2) x depth-quarters(4) = 8 shards; each core owns a 32-deep
output slab (+4-voxel halo for the 9^3 box filter).

NCC box filter strategy (per var in {I, J, I^2, J^2, IJ}):
  pass1 (PE):  per d-row matmul(lhsT=V_d[h,w], rhs=BandH[h,h']) -> PSUM [w, h]
               = H-box + transpose in one shot (fp16, FD=128, LDW pipelined)
  evac1:       PSUM -> SBUF fp16 YT [w, (d, h)]
  pass2 (PE):  stationary BandW (scaled); 3 d-shifted FD=512 matmuls accumulate
               -> t3[r] = Z[r]+Z[r+1]+Z[r+2] (W-box + D-triple), PSUM
  evac2:       PSUM -> SBUF fp16 T3 [w', (r, h)]
  D-final(DVE): S = t3[d] + t3[d+3] + t3[d+6]  (9-window box done)

Scaling: quadratic vars (II/JJ/IJ) get BandW*s (s=fp16(1/27)); linear vars
(I/J) get BandW*t with t^2 = s/729, so cross = B'IJ - B'I*B'J and
var = B'II - B'I^2 need no scalar coefficient (all plain tensor ops), and
everything stays in fp16 range. eps scales as 1e-5*s^2.

The depth range is processed in two phases (A: do 0..15, B: do 16..31) so the
cc math for A overlaps phase B's box-filter work.
"""

import os
import sys

for _p in ("/opt/trn_rl_repo",):
    if _p not in sys.path:
        sys.path.insert(0, _p)

import numpy as np

import concourse.bass as bass
import concourse.tile as tile
from concourse import bacc, mybir
from concourse.bass_utils import run_bass_kernel_spmd

F32 = mybir.dt.float32
F16 = mybir.dt.float16
AF = mybir.ActivationFunctionType
ALU = mybir.AluOpType

HP = 128
W = 128
D_FULL = 128
DQ = 32
D_IN = DQ + 8     # 40 slab rows incl halo
YT_R = 42         # yt rows incl zero tail (pass2 k=9 s=2 reads rows 38..41)
T3_R = 40
FLOW_D = DQ + 1   # 33

N_IN = D_IN * W           # 5120
N_YT = YT_R * HP          # 5376
N_T3 = T3_R * HP          # 5120
N_BOX = DQ * HP           # 4096
N_RECON = DQ * W          # 4096
N_FLOW_C = FLOW_D * W     # 4224

S16 = float(np.float16(1.0 / 27.0))          # quadratic-var scale
TLIN = float(np.float16(np.sqrt(S16 / 729.0)))  # linear-var scale
EPS_S = 1e-5 * S16 * S16

COL_CC = 0     # +2
COL_MSE = 2
COL_DX = 3     # +3
COL_DZ = 6     # +3
COL_DY = 9     # +12
ACC_W = 24

VARS = ("J", "I", "II", "JJ", "IJ")

_CACHE = {}


def _patch_act_tables():
    """Reorder activation-table sets so the one containing ln+exp+square+copy
    is preferred, avoiding table reloads between Ln and Exp/Square."""
    from concourse import hw_specs

    if getattr(hw_specs, "_gvsl_patched", False):
        return
    orig = hw_specs.get_activation_tables

    def patched(arch):
        t = dict(orig(arch))
        key = "natural_log_exp_and_others"
        if key in t:
            t = {key: t[key], **{k: v for k, v in t.items() if k != key}}
        return t

    hw_specs.get_activation_tables = patched
    bacc.get_activation_tables = patched
    hw_specs._gvsl_patched = True


def _build_program():
    nc = bacc.Bacc("TRN2", target_bir_lowering=False, debug=False, num_devices=8)

    d_inI = nc.dram_tensor("inI", [HP, N_IN], F16, kind="ExternalInput").ap()
    d_inJ = nc.dram_tensor("inJ", [HP, N_IN], F16, kind="ExternalInput").ap()
    d_recon = nc.dram_tensor("recon", [HP, N_RECON], F16, kind="ExternalInput").ap()
    d_flow = nc.dram_tensor("flow", [HP, 3 * N_FLOW_C], F16, kind="ExternalInput").ap()
    d_bandh = nc.dram_tensor("bandh", [HP, HP], F16, kind="ExternalInput").ap()
    d_bandq = nc.dram_tensor("bandq", [HP, HP], F16, kind="ExternalInput").ap()
    d_bandl = nc.dram_tensor("bandl", [HP, HP], F16, kind="ExternalInput").ap()
    d_bidiag = nc.dram_tensor("bidiag", [HP, HP - 1], F16, kind="ExternalInput").ap()
    d_out = nc.dram_tensor("out", [HP, ACC_W], F32, kind="ExternalOutput").ap()

    from contextlib import ExitStack

    with tile.TileContext(nc) as tc, ExitStack() as es:
        pp = es.enter_context(tc.tile_pool(name="persist", bufs=1))
        prp = es.enter_context(tc.tile_pool(name="prodp", bufs=2))
        ytp = es.enter_context(tc.tile_pool(name="ytp", bufs=2))
        t3p = es.enter_context(tc.tile_pool(name="t3p", bufs=2))
        bxp = es.enter_context(tc.tile_pool(name="boxp", bufs=1))
        scp = es.enter_context(tc.tile_pool(name="scrp", bufs=2))
        flp = es.enter_context(tc.tile_pool(name="flowscr", bufs=2))
        ps1 = es.enter_context(tc.tile_pool(name="psum1", bufs=2, space="PSUM"))
        ps2 = es.enter_context(tc.tile_pool(name="psum2", bufs=2, space="PSUM"))

        acc = pp.tile([HP, ACC_W], F32, tag="acc", name="acc")[:]
        eps_ap = pp.tile([HP, 1], F32, tag="epsc", name="epsc")[:]
        nc.gpsimd.memset(eps_ap, EPS_S)

        bandh = pp.tile([HP, HP], F16, tag="bandh", name="bandh")[:]
        bandq = pp.tile([HP, HP], F16, tag="bandq", name="bandq")[:]
        bandl = pp.tile([HP, HP], F16, tag="bandl", name="bandl")[:]
        bidiag = pp.tile([HP, HP - 1], F16, tag="bidiag", name="bidiag")[:]
        inI = pp.tile([HP, N_IN], F16, tag="inI", name="inI")[:]
        inJ = pp.tile([HP, N_IN], F16, tag="inJ", name="inJ")[:]
        recon = pp.tile([HP, N_RECON], F16, tag="recon", name="recon")[:]
        flow = pp.tile([HP, 3 * N_FLOW_C], F16, tag="flow", name="flow")[:]

        nc.sync.dma_start(out=bandh, in_=d_bandh)
        nc.sync.dma_start(out=bandq, in_=d_bandq)
        nc.sync.dma_start(out=bandl, in_=d_bandl)
        nc.sync.dma_start(out=bidiag, in_=d_bidiag)
        NQ = N_IN // 4
        for c in range(4):
            nc.sync.dma_start(out=inJ[:, NQ * c : NQ * (c + 1)],
                              in_=d_inJ[:, NQ * c : NQ * (c + 1)])
        for c in range(4):
            nc.sync.dma_start(out=inI[:, NQ * c : NQ * (c + 1)],
                              in_=d_inI[:, NQ * c : NQ * (c + 1)])
        nc.sync.dma_start(out=recon, in_=d_recon)
        nc.sync.dma_start(out=flow, in_=d_flow)

        inI_r = inI.rearrange("p (d w) -> p d w", w=W)
        inJ_r = inJ.rearrange("p (d w) -> p d w", w=W)

        evac_ct = [0]

        def evac(dst, src):
            # alternate PSUM->SBUF copies between DVE and ACT
            if evac_ct[0] % 2 == 0:
                nc.vector.tensor_copy(dst, src)
            else:
                nc.scalar.copy(dst, src)
            evac_ct[0] += 1

        def product(v):
            prod = prp.tile([HP, N_IN], F16, tag="prod", name=f"prod{v}")[:]
            if v == "II":
                nc.vector.tensor_mul(prod, inI, inI)
            elif v == "JJ":
                nc.vector.tensor_mul(prod, inJ, inJ)
            else:
                nc.vector.tensor_mul(prod, inI, inJ)
            return prod.rearrange("p (d w) -> p d w", w=W)

        def ncc_var(v, src_r):
            # pass1: H-box + transpose, 8 d-rows per psum tile
            ytt = ytp.tile([HP, N_YT], F16, tag="yt", name=f"yt{v}")[:]
            yt_r = ytt.rearrange("p (d h) -> p d h", h=HP)
            nc.gpsimd.memset(yt_r[:, D_IN:YT_R, :], 0.0)
            for g0 in range(0, D_IN, 8):
                pst = ps1.tile([HP, 1024], F32, tag="ps1", name="ps1")[:]
                for q in range(8):
                    nc.tensor.matmul(
                        pst[:, 128 * q : 128 * (q + 1)],
                        src_r[:, g0 + q, :],
                        bandh,
                        start=True,
                        stop=True,
                    )
                dst = yt_r[:, g0 : g0 + 8, :].rearrange("p d h -> p (d h)")
                evac(dst, pst)

            # pass2: W-box (scaled) + D-triple, 2 k-tiles per psum tile
            bw = bandl if v in ("I", "J") else bandq
            t3t = t3p.tile([HP, N_T3], F16, tag="t3", name=f"t3{v}")[:]
            t3_r = t3t.rearrange("p (r h) -> p r h", h=HP)
            for k0 in range(0, 10, 2):
                pst = ps2.tile([HP, 1024], F32, tag="ps2", name="ps2")[:]
                for ki in range(2):
                    k = k0 + ki
                    for s in range(3):
                        rhs = yt_r[:, 4 * k + s : 4 * k + s + 4, :].rearrange(
                            "p d h -> p (d h)"
                        )
                        nc.tensor.matmul(
                            pst[:, 512 * ki : 512 * (ki + 1)],
                            bw,
                            rhs,
                            start=(s == 0),
                            stop=(s == 2),
                        )
                dst = t3_r[:, 4 * k0 : 4 * k0 + 8, :].rearrange("p r h -> p (r h)")
                evac(dst, pst)

            # D-final: S[do] = t3[do] + t3[do+3] + t3[do+6]
            B = bxp.tile([HP, N_BOX], F16, tag=f"box{v}", name=f"box{v}")[:]
            B_r = B.rearrange("p (do h) -> p do h", h=HP)
            nc.vector.tensor_add(B_r, t3_r[:, 0:DQ, :], t3_r[:, 3 : 3 + DQ, :])
            nc.vector.tensor_add(B_r, B_r, t3_r[:, 6 : 6 + DQ, :])
            return B

        flow_r = flow.rearrange("p (c d w) -> p c d w", c=3, w=W)

        def flow_dy(c):
            fc_flat = flow_r[:, c].rearrange("p d w -> p (d w)")
            for half in range(2):
                pst = ps1.tile([HP, 1024], F32, tag="ps1", name="dy")[:]
                for j in range(2):
                    off = 2048 * half + 1024 * j
                    for jj in range(2):
                        nc.tensor.matmul(
                            pst[0 : HP - 1, 512 * jj : 512 * (jj + 1)],
                            bidiag,
                            fc_flat[:, off + 512 * jj : off + 512 * (jj + 1)],
                            start=True,
                            stop=True,
                        )
                    col = COL_DY + 4 * c + 2 * half + j
                    nc.scalar.activation(
                        pst[0 : HP - 1, :],
                        pst[0 : HP - 1, :],
                        AF.Square,
                        accum_out=acc[0 : HP - 1, col : col + 1],
                    )

        def flow_dxz(c):
            fc_flat = flow_r[:, c].rearrange("p d w -> p (d w)")
            db = flp.tile([HP, N_RECON], F16, tag="fscr", name=f"dx{c}")[:]
            nc.vector.tensor_sub(
                db, fc_flat[:, 1 : 1 + N_RECON], fc_flat[:, 0:N_RECON]
            )
            db_r = db.rearrange("p (d w) -> p d w", w=W)
            nc.scalar.activation(
                db_r[:, :, 0 : W - 1],
                db_r[:, :, 0 : W - 1],
                AF.Square,
                accum_out=acc[:, COL_DX + c : COL_DX + c + 1],
            )
            db2 = flp.tile([HP, N_RECON], F16, tag="fscr", name=f"dz{c}")[:]
            nc.vector.tensor_sub(
                db2, fc_flat[:, W : W + N_RECON], fc_flat[:, 0:N_RECON]
            )
            nc.scalar.activation(
                db2, db2, AF.Square, accum_out=acc[:, COL_DZ + c : COL_DZ + c + 1]
            )

        def mse():
            md = flp.tile([HP, N_RECON], F16, tag="fscr", name="mse")[:]
            nc.vector.tensor_sub(
                md,
                inJ_r[:, 4 : 4 + DQ, :].rearrange("p d w -> p (d w)"),
                recon,
            )
            nc.scalar.activation(
                md, md, AF.Square, accum_out=acc[:, COL_MSE : COL_MSE + 1]
            )

        boxes = {}
        for v in VARS:
            src_r = {"J": inJ_r, "I": inI_r}.get(v)
            if src_r is None:
                src_r = product(v)
            boxes[v] = ncc_var(v, src_r)
            # independent flow/mse work interleaved to fill engine gaps
            if v == "J":
                mse()
            elif v == "I":
                flow_dy(0)
                flow_dxz(0)
            elif v == "II":
                flow_dy(1)
                flow_dxz(1)
            elif v == "JJ":
                flow_dy(2)
                flow_dxz(2)

        # cc math on the full 32-depth boxes, 2 slices, ACT functions batched
        NS = N_BOX // 2
        sbufs = []
        for sl in range(2):
            lo, hi = sl * NS, (sl + 1) * NS
            s1 = scp.tile([HP, NS], F16, tag="s1", name="s1")[:]
            s2 = scp.tile([HP, NS], F16, tag="s2", name="s2")[:]
            s3 = scp.tile([HP, NS], F16, tag="s3", name="s3")[:]
            bi, bj = boxes["I"][:, lo:hi], boxes["J"][:, lo:hi]
            bii, bjj = boxes["II"][:, lo:hi], boxes["JJ"][:, lo:hi]
            bij = boxes["IJ"][:, lo:hi]

            nc.vector.tensor_mul(s1, bi, bj)
            nc.vector.tensor_sub(s2, bij, s1)        # crossS
            nc.vector.tensor_mul(s1, bi, bi)
            nc.vector.tensor_sub(s3, bii, s1)        # IvarS
            nc.vector.tensor_mul(s1, bj, bj)
            nc.vector.tensor_sub(s1, bjj, s1)        # JvarS
            nc.vector.tensor_mul(s1, s1, s3)         # denomS
            nc.scalar.activation(s3, s2, AF.Square)  # crossS^2
            sbufs.append((s1, s2, s3))
        for sl in range(2):
            s1, s2, s3 = sbufs[sl]
            nc.scalar.activation(s2, s3, AF.Ln)      # ln cross^2
            nc.scalar.activation(s3, s1, AF.Ln, bias=eps_ap)  # ln(denom+eps)
        for sl in range(2):
            s1, s2, s3 = sbufs[sl]
            nc.vector.tensor_sub(s1, s2, s3)
            col = COL_CC + sl
            nc.scalar.activation(
                s3, s1, AF.Exp, accum_out=acc[:, col : col + 1]
            )

        nc.sync.dma_start(out=d_out, in_=acc)

    nc.compile()
    return nc


def _make_consts():
    k = np.arange(HP)
    band = (np.abs(k[:, None] - k[None, :]) <= 4).astype(np.float16)
    bandq = (band * np.float16(S16)).astype(np.float16)
    bandl = (band * np.float16(TLIN)).astype(np.float16)
    m = np.arange(HP - 1)
    bidiag = np.zeros((HP, HP - 1), np.float16)
    bidiag[m + 1, m] = 1.0
    bidiag[m, m] = -1.0
    return band, bandq, bandl, bidiag


def _shard_inputs(imgsA, recon_A, warped_BA, flow_BA):
    bandh, bandq, bandl, bidiag = _make_consts()
    in_maps = []
    for core in range(8):
        b, q = divmod(core, 4)
        d0 = DQ * q

        def slab(vol):
            s = np.zeros((HP, D_IN, W), np.float16)
            lo, hi = d0 - 4, d0 + DQ + 4
            clo, chi = max(lo, 0), min(hi, D_FULL)
            s[:, clo - lo : chi - lo, :] = vol[clo:chi].transpose(1, 0, 2)
            return s.reshape(HP, N_IN)

        rec = (
            recon_A[b, 0, d0 : d0 + DQ]
            .transpose(1, 0, 2)
            .astype(np.float16)
            .reshape(HP, N_RECON)
        )

        fl = np.empty((HP, 3, FLOW_D, W), np.float16)
        hi = min(d0 + FLOW_D, D_FULL)
        n = hi - d0
        fl[:, :, :n] = flow_BA[b, :, d0:hi].transpose(2, 0, 1, 3)
        if n < FLOW_D:
            fl[:, :, n:] = fl[:, :, n - 1 : n]

        in_maps.append(
            {
                "inI": slab(warped_BA[b, 0]),
                "inJ": slab(imgsA[b, 0]),
                "recon": np.ascontiguousarray(rec),
                "flow": np.ascontiguousarray(fl).reshape(HP, 3 * N_FLOW_C),
                "bandh": bandh,
                "bandq": bandq,
                "bandl": bandl,
                "bidiag": bidiag,
            }
        )
    return in_maps


def _install_profile_shim():
    """Wire up NTFF profiling under axon when antenv.axon_hooks is absent."""
    try:
        import antenv.axon_hooks  # noqa: F401

        return True
    except ImportError:
        pass
    import contextlib
    import ctypes
    import types

    so_path = "/opt/axon/libaxon_pjrt.so"
    if not os.path.exists(so_path):
        return False
    lib = ctypes.CDLL(so_path)
    if not hasattr(lib, "axon_start_nrt_profile"):
        return False
    lib.axon_start_nrt_profile.argtypes = [
        ctypes.POINTER(ctypes.c_int64),
        ctypes.c_size_t,
    ]
    lib.axon_start_nrt_profile.restype = ctypes.c_int64
    lib.axon_stop_nrt_profile.argtypes = [ctypes.c_char_p]
    lib.axon_stop_nrt_profile.restype = ctypes.c_int64

    @contextlib.contextmanager
    def _hook(output_dir, device_ids):
        import jax

        jax.devices()
        if device_ids:
            ids = (ctypes.c_int64 * len(device_ids))(*device_ids)
            rc = lib.axon_start_nrt_profile(ids, len(device_ids))
        else:
            rc = lib.axon_start_nrt_profile(None, 0)
        if rc != 0:
            raise RuntimeError(f"axon_start_nrt_profile rc={rc}")
        try:
            yield
        finally:
            n = lib.axon_stop_nrt_profile(str(output_dir).encode())
            print(f"ntff profile: {n} file(s) written to {output_dir}")

    mod = types.ModuleType("antenv.axon_hooks")
    mod.get_axon_ntff_profile_hook = lambda: _hook
    mod.set_axon_ntff_profile_hook = lambda h: None
    import antenv

    sys.modules["antenv.axon_hooks"] = mod
    antenv.axon_hooks = mod

    import concourse.bass_utils as _bu

    _bu.upload_artifacts = lambda tmpdir: tmpdir
    return True


LAST_EXEC_NS = None
LAST_RESULTS = None


def kernel(imgsA, recon_A, warped_BA, flow_BA):
    global LAST_EXEC_NS, LAST_RESULTS
    if "nc" not in _CACHE:
        _CACHE["nc"] = _build_program()
    nc = _CACHE["nc"]

    in_maps = _shard_inputs(
        np.asarray(imgsA, np.float32),
        np.asarray(recon_A, np.float32),
        np.asarray(warped_BA, np.float32),
        np.asarray(flow_BA, np.float32),
    )
    trace = os.environ.get("GVSL_TRACE", "0") == "1"
    if trace:
        trace = _install_profile_shim()
    tmpdir = os.environ.get("GVSL_TRACE_DIR") or None
    res = run_bass_kernel_spmd(
        nc, in_maps, core_ids=list(range(8)), trace=trace, tmpdir=tmpdir
    )
    LAST_EXEC_NS = res.exec_time_ns
    LAST_RESULTS = res

    cc = mse_s = dx = dy = dz = 0.0
    for r in res.results:
        o = np.asarray(r["out"], np.float64)
        cc += o[:, COL_CC : COL_CC + 2].sum()
        mse_s += o[:, COL_MSE].sum()
        dx += o[:, COL_DX : COL_DX + 3].sum()
        dz += o[:, COL_DZ : COL_DZ + 3].sum()
        dy += o[: HP - 1, COL_DY : COL_DY + 12].sum()

    if os.environ.get("GVSL_DEBUG_COLS"):
        tot = np.zeros(ACC_W)
        for r in res.results:
            tot += np.asarray(r["out"], np.float64).sum(axis=0)
        n_dd = 2.0 * 127 * 128 * 128
        print("cols cc:", tot[COL_CC : COL_CC + 2])
        print("col mse:", tot[COL_MSE])
        print("cols dx/nd:", tot[COL_DX : COL_DX + 3] / n_dd * 3)
        print("cols dz/nd:", tot[COL_DZ : COL_DZ + 3] / n_dd * 3)
        print("cols dy/nd:", tot[COL_DY : COL_DY + 12].reshape(3, 4) / n_dd * 3)

    n_vox = 2 * 1 * 128 * 128 * 128
    n_d = 2 * 3 * 127 * 128 * 128
    ncc_loss = 1.0 - cc / n_vox
    mse_loss = mse_s / n_vox
    smooth_loss = (dx / n_d + dy / n_d + dz / n_d) / 3.0
    return (
        np.float32(ncc_loss),
        np.float32(mse_loss),
        np.float32(smooth_loss),
    )
